# revision 1
# baseline (speedup 1.0000x reference)
"""Trainium2 Bass kernel: dense attention with key-padding mask (ColoAttention).

Math (per batch b, head h):
    scores = (Q @ K^T) / sqrt(D); masked keys -> -inf; softmax over keys;
    out = probs @ V; rows at masked query positions zeroed.

Implementation notes:
  - K and V rows at masked key positions are zeroed on the host.  Then
    scores at masked keys are exactly 0, exp(0) = 1, and the per-row sum of
    exponentials just needs the (host-known) masked-key count subtracted.
    Masked keys contribute 0 to probs @ V since their V rows are zero.
  - Scores are computed transposed (S^T[k, q] = K @ Q^T) so the exp output
    P^T[k, q] (bf16) is directly the moving operand for O'^T = V^T @ P^T.
  - Row sums come from a ones-vector matmul over P^T.  The device emits the
    unnormalized O'^T and raw sums; the host applies qmask/(sums-mcount) and
    the final [D,S]->[S,D] transpose while scattering into the full output.
  - QK^T runs in float32r (full-rate fp32 path on the PE), PV in bf16.
  - Sharding: B*H = 64 (b,h) pairs; core c handles batch c//2, heads
    (c%2)*8 .. +8.  Pure SPMD, no collectives.
"""

import numpy as np
import ml_dtypes
from contextlib import ExitStack

import concourse.bass as bass
import concourse.mybir as mybir
import concourse.tile as tile
from concourse import bacc
from concourse.bass_utils import run_bass_kernel_spmd
from concourse.masks import make_identity

B, S, H, D = 4, 2048, 16, 128
N_CORES = 8
CORES_PER_BATCH = N_CORES // B            # 2
HPC = H // CORES_PER_BATCH                # 8 heads (pairs) per core
P = 128
SCALE = 1.0 / float(np.sqrt(np.float64(D)).astype(np.float32))


def build_program(n_pairs: int = HPC, seq: int = S) -> bacc.Bacc:
    KCN = seq // P          # key chunks of 128
    NW = seq // 512         # 512-wide q windows
    QTN = seq // P          # q tiles of 128
    f32 = mybir.dt.float32
    f32r = mybir.dt.float32r
    bf16 = mybir.dt.bfloat16
    Exp = mybir.ActivationFunctionType.Exp

    nc = bacc.Bacc("TRN2", target_bir_lowering=False, debug=False)
    # q/k arrive pre-transposed from the host: [pair, D, S]
    q_d = nc.dram_tensor("q", [n_pairs, P, seq], f32r, kind="ExternalInput").ap()
    k_d = nc.dram_tensor("k", [n_pairs, P, seq], f32r, kind="ExternalInput").ap()
    v_d = nc.dram_tensor("v", [n_pairs, seq, P], bf16, kind="ExternalInput").ap()
    qmask_d = nc.dram_tensor("qmask", [seq], f32, kind="ExternalInput").ap()
    mcnt_d = nc.dram_tensor("mcount", [P, 1], f32, kind="ExternalInput").ap()
    out_d = nc.dram_tensor("out", [n_pairs, P, seq], f32, kind="ExternalOutput").ap()
    sums_d = nc.dram_tensor("sums_out", [n_pairs, seq], f32, kind="ExternalOutput").ap()

    # q is processed in halves of QH columns; scores PSUM double-buffered so
    # QK(kc+1) overlaps exp(kc); ones-matmul row sums interleave per kc.
    QH = min(seq, 1024)      # q-half width
    NH = seq // QH           # number of halves
    NWH = QH // 512          # 512-wide windows per half
    QTH = QH // P            # 128-wide q tiles per half

    with tile.TileContext(nc) as tc:
        with ExitStack() as ctx:
            consts = ctx.enter_context(tc.tile_pool(name="consts", bufs=1))
            inp = ctx.enter_context(tc.tile_pool(name="inp", bufs=2))
            qtp = ctx.enter_context(tc.tile_pool(name="qtp", bufs=2))
            ptp = ctx.enter_context(tc.tile_pool(name="ptp", bufs=2))
            otp = ctx.enter_context(tc.tile_pool(name="otp", bufs=4))
            outp = ctx.enter_context(tc.tile_pool(name="outp", bufs=4))
            smp = ctx.enter_context(tc.tile_pool(name="smp", bufs=2))
            # PSUM (8 banks): "s" scores [128,QH] x2 bufs = 4 banks,
            # "o" O' accum [128,512] x2 = 2 banks (also O transposes),
            # "sm" row sums [1,QH] = 2 banks.
            sps = ctx.enter_context(tc.tile_pool(name="sps", bufs=2, space="PSUM"))
            ops = ctx.enter_context(tc.tile_pool(name="ops", bufs=2, space="PSUM"))
            smps = ctx.enter_context(tc.tile_pool(name="smps", bufs=1, space="PSUM"))

            ones_b = consts.tile([P, 1], bf16, tag="ones")
            nc.gpsimd.memset(ones_b[:], 1.0)

            for p in range(n_pairs):
                # ---- stage inputs (q/k already transposed on host) ----
                qt_sb = qtp.tile([P, seq], f32r, tag="qt")
                nc.sync.dma_start(qt_sb[:], q_d[p])
                kt_sb = qtp.tile([P, seq], f32r, tag="kt")
                nc.sync.dma_start(kt_sb[:], k_d[p])
                v_sb = inp.tile([P, KCN, P], bf16, tag="v_sb")
                nc.sync.dma_start(v_sb[:], v_d[p].rearrange("(t r) d -> r t d", r=P))

                # ---- scores + exp + PV + row sums, software-pipelined so
                # PE always has QK(step+1) to run while ACT does exp(step).
                def emit_qk(h, kc, tag_i):
                    s_ps = sps.tile([P, QH], f32, tag="s", name=f"s_{p}_{h}_{kc}")
                    for w in range(NWH):
                        nc.tensor.matmul(
                            s_ps[:, w * 512:(w + 1) * 512],
                            lhsT=kt_sb[:, kc * P:(kc + 1) * P],
                            rhs=qt_sb[:, h * QH + w * 512:h * QH + (w + 1) * 512],
                            start=True, stop=True)
                    return s_ps

                steps = [(h, kc) for h in range(NH) for kc in range(KCN)]
                pt_half = {}
                ot_half = {}
                sm_half = {}
                pend = {0: emit_qk(*steps[0], 0)}
                for i, (h, kc) in enumerate(steps):
                    q0 = h * QH
                    if h not in pt_half:
                        pt_half[h] = ptp.tile([P, KCN, QH], bf16, tag="pt",
                                              name=f"pt_{p}_{h}")
                        ot_half[h] = [
                            ops.tile([P, 512], f32, tag="o", name=f"ot_{p}_{h}_{w}")
                            for w in range(NWH)]
                        sm_half[h] = smps.tile([1, QH], f32, tag="sm",
                                               name=f"sm_{p}_{h}")
                    pt_sb, ot_ps, sm_ps = pt_half[h], ot_half[h], sm_half[h]
                    s_ps = pend.pop(i)
                    nc.scalar.activation(pt_sb[:, kc, :], s_ps[:], Exp, scale=SCALE)
                    if i + 1 < len(steps):
                        pend[i + 1] = emit_qk(*steps[i + 1], i + 1)
                    for w in range(NWH):
                        nc.tensor.matmul(
                            ot_ps[w][:],
                            lhsT=v_sb[:, kc, :],
                            rhs=pt_sb[:, kc, w * 512:(w + 1) * 512],
                            start=(kc == 0), stop=(kc == KCN - 1))
                    for w in range(NWH):
                        nc.tensor.matmul(
                            sm_ps[0:1, w * 512:(w + 1) * 512],
                            lhsT=ones_b[:],
                            rhs=pt_sb[:, kc, w * 512:(w + 1) * 512],
                            start=(kc == 0), stop=(kc == KCN - 1))
                    if kc != KCN - 1:
                        continue

                    # ---- half tail: store O'^T and raw sums; host does
                    # the normalize + final transpose ----
                    for w in range(NWH):
                        o_stage = otp.tile([P, 512], f32, tag="otsb",
                                           name=f"otsb_{p}_{h}_{w}")
                        nc.vector.tensor_copy(out=o_stage[:], in_=ot_ps[w][:])
                        nc.sync.dma_start(
                            out_d[p, :, q0 + w * 512:q0 + (w + 1) * 512],
                            o_stage[:])
                    sm_sb = smp.tile([1, QH], f32, tag="sm_sb")
                    nc.vector.tensor_copy(out=sm_sb[:], in_=sm_ps[:])
                    nc.sync.dma_start(sums_d[p, q0:q0 + QH], sm_sb[:])

    nc.compile()
    return nc


_PROG_CACHE: dict = {}


def _get_program() -> bacc.Bacc:
    if "nc" not in _PROG_CACHE:
        _PROG_CACHE["nc"] = build_program(HPC, S)
    return _PROG_CACHE["nc"]


def make_in_maps(query, key, value, attn_mask):
    # device wants q/k as [pair, D, S] (pre-transposed), v as [pair, S, D]
    qT = np.ascontiguousarray(np.asarray(query, np.float32).transpose(0, 2, 3, 1))
    kT = np.asarray(key, np.float32).transpose(0, 2, 3, 1)       # [B, H, D, S]
    v = np.asarray(value, np.float32).transpose(0, 2, 1, 3)      # [B, H, S, D]
    mf = (np.asarray(attn_mask) > 0).astype(np.float32)          # [B, S]
    kTz = np.ascontiguousarray(kT * mf[:, None, None, :])
    vz = (v * mf[:, None, :, None]).astype(ml_dtypes.bfloat16)
    mcount = (S - mf.sum(axis=1)).astype(np.float32)             # [B]
    in_maps = []
    for c in range(N_CORES):
        b, h0 = c // CORES_PER_BATCH, (c % CORES_PER_BATCH) * HPC
        in_maps.append({
            "q": np.ascontiguousarray(qT[b, h0:h0 + HPC]),
            "k": np.ascontiguousarray(kTz[b, h0:h0 + HPC]),
            "v": np.ascontiguousarray(vz[b, h0:h0 + HPC]),
            "qmask": mf[b],
            "mcount": np.full((P, 1), mcount[b], np.float32),
        })
    return in_maps, mf


def assemble_output(results, mf):
    out = np.empty((B, S, H * D), np.float32)
    mcount = (S - mf.sum(axis=1)).astype(np.float32)
    for c in range(N_CORES):
        b, h0 = c // CORES_PER_BATCH, (c % CORES_PER_BATCH) * HPC
        oT = results[c]["out"]                                   # [HPC, D, S]
        sums = results[c]["sums_out"] - mcount[b]                # [HPC, S]
        with np.errstate(divide="ignore", invalid="ignore"):
            scale = np.where(mf[b][None, :] > 0, 1.0 / sums, 0.0)
        o = oT * scale[:, None, :]                               # [HPC, D, S]
        for i in range(HPC):
            out[b, :, (h0 + i) * D:(h0 + i + 1) * D] = o[i].T
    for b in range(B):
        if mf[b].sum() == 0.0:                                   # degenerate batch
            out[b] = 0.0
    return out


def kernel(query, key, value, attn_mask):
    nc = _get_program()
    in_maps, mf = make_in_maps(query, key, value, attn_mask)
    res = run_bass_kernel_spmd(nc, in_maps, list(range(N_CORES)))
    return assemble_output(res.results, mf)



# revision 3
# speedup vs baseline: 1.5503x; 1.5503x over previous
"""Trainium2 Bass kernel: dense attention with key-padding mask (ColoAttention).

Math (per batch b, head h):
    scores = (Q @ K^T) / sqrt(D); masked keys -> -inf; softmax over keys;
    out = probs @ V; rows at masked query positions zeroed.

Implementation notes (v2):
  - K and V rows at masked key positions are zeroed on the host, so masked
    scores are exactly 0, exp(0) = 1, and the host subtracts the masked-key
    count from each row's sum of exponentials.  Masked V rows contribute 0.
  - The mask is a contiguous valid prefix; the host reads max_len from it and
    trims compute to NKC = ceil(max_len/128) key chunks and NKC*128 query
    columns (identical on every core, so the single SPMD program stays
    static).  Queries/keys beyond that are masked for every batch and are
    repadded with zeros on the host.
  - Scores are computed transposed (S^T[k, q] = K @ Q^T) so the exp output
    P^T (bf16) directly feeds O'^T = V^T @ P^T.
  - Row sums: DVE accumulates P^T chunks elementwise in bf16 (4x perf mode),
    then a single ones-vector matmul per q-half does the 128-way cross-
    partition reduction.  This keeps the PE free for QK^T / PV only.
  - QK^T runs in float32r (full-rate fp32), PV in bf16.  The device emits
    unnormalized O'^T and raw sums; the host normalizes and transposes while
    scattering into the full output.
  - Sharding: B*H = 64 (b,h) pairs; core c handles batch c//2, heads
    (c%2)*8 .. +8.  Pure SPMD, no collectives.
"""

import numpy as np
import ml_dtypes
from contextlib import ExitStack

import concourse.bass as bass
import concourse.mybir as mybir
import concourse.tile as tile
from concourse import bacc
from concourse.bass_utils import run_bass_kernel_spmd

B, S, H, D = 4, 2048, 16, 128
N_CORES = 8
CORES_PER_BATCH = N_CORES // B            # 2
HPC = H // CORES_PER_BATCH                # 8 (b,h) pairs per core
P = 128
SCALE = 1.0 / float(np.sqrt(np.float64(D)).astype(np.float32))


def build_program(n_pairs: int = HPC, nkc: int = 16) -> bacc.Bacc:
    """One core's program: n_pairs heads, nkc key chunks of 128, q range
    nkc*128 split into 2 halves of 2 windows each."""
    W = nkc * 32            # q window width (>=256 for full-rate f32r)
    QH = 2 * W              # q columns per half
    LQ = nkc * 128          # total q columns
    LK = nkc * 128          # total keys
    f32 = mybir.dt.float32
    f32r = mybir.dt.float32r
    bf16 = mybir.dt.bfloat16
    Exp = mybir.ActivationFunctionType.Exp
    Add = mybir.AluOpType.add

    nc = bacc.Bacc("TRN2", target_bir_lowering=False, debug=False)
    # q/k arrive pre-transposed from the host: [pair, D, S-trimmed]
    q_d = nc.dram_tensor("q", [n_pairs, P, LQ], f32r, kind="ExternalInput").ap()
    k_d = nc.dram_tensor("k", [n_pairs, P, LK], f32r, kind="ExternalInput").ap()
    v_d = nc.dram_tensor("v", [n_pairs, LK, P], bf16, kind="ExternalInput").ap()
    out_d = nc.dram_tensor("out", [n_pairs, P, LQ], f32, kind="ExternalOutput").ap()
    sums_d = nc.dram_tensor("sums_out", [n_pairs, LQ], f32, kind="ExternalOutput").ap()

    with tile.TileContext(nc) as tc:
        with ExitStack() as ctx:
            consts = ctx.enter_context(tc.tile_pool(name="consts", bufs=1))
            qtp = ctx.enter_context(tc.tile_pool(name="qtp", bufs=2))
            inp = ctx.enter_context(tc.tile_pool(name="inp", bufs=2))
            ptp = ctx.enter_context(tc.tile_pool(name="ptp", bufs=3))
            accp = ctx.enter_context(tc.tile_pool(name="accp", bufs=2))
            otp = ctx.enter_context(tc.tile_pool(name="otp", bufs=4))
            smp = ctx.enter_context(tc.tile_pool(name="smp", bufs=2))
            # PSUM (8 banks): scores 2 bufs x [128,2,512] f32 = 4 banks,
            # O' accum 2 x [128,W] = 2 banks, sums [1,2,512] = 2 banks.
            sps = ctx.enter_context(tc.tile_pool(name="sps", bufs=2, space="PSUM"))
            ops = ctx.enter_context(tc.tile_pool(name="ops", bufs=2, space="PSUM"))
            smps = ctx.enter_context(tc.tile_pool(name="smps", bufs=1, space="PSUM"))

            ones_b = consts.tile([P, 1], bf16, tag="ones")
            nc.gpsimd.memset(ones_b[:], 1.0)

            pair_tiles = {}

            def load_pair(p):
                qt = qtp.tile([P, LQ], f32r, tag="qt", name=f"qt_{p}")
                nc.sync.dma_start(qt[:], q_d[p])
                kt = qtp.tile([P, LK], f32r, tag="kt", name=f"kt_{p}")
                nc.sync.dma_start(kt[:], k_d[p])
                v = inp.tile([P, nkc, P], bf16, tag="v", name=f"v_{p}")
                nc.sync.dma_start(v[:], v_d[p].rearrange("(t r) d -> r t d", r=P))
                pair_tiles[p] = (qt, kt, v)

            gsteps = [(p, h, kc)
                      for p in range(n_pairs) for h in (0, 1)
                      for kc in range(nkc)]

            def emit_qk(p, h, kc, i):
                if p not in pair_tiles:
                    load_pair(p)
                qt, kt, v = pair_tiles[p]
                s = sps.tile([P, 2, 512], f32, tag="s", name=f"s_{i}")
                for w in (0, 1):
                    nc.tensor.matmul(
                        s[:, w, 0:W],
                        lhsT=kt[:, kc * P:(kc + 1) * P],
                        rhs=qt[:, h * QH + w * W: h * QH + (w + 1) * W],
                        start=True, stop=True)
                return s

            half_state = {}
            pend = {0: emit_qk(*gsteps[0], 0)}
            for i, (p, h, kc) in enumerate(gsteps):
                qt, kt, v = pair_tiles[p]
                if (p, h) not in half_state:
                    half_state[(p, h)] = (
                        [ops.tile([P, W], f32, tag="o", name=f"o_{p}_{h}_{w}")
                         for w in (0, 1)],
                        accp.tile([P, 2, W], bf16, tag="acc", name=f"acc_{p}_{h}"),
                        smps.tile([1, 2, 512], f32, tag="sm", name=f"sm_{p}_{h}"),
                    )
                o_ps, acc, sm_ps = half_state[(p, h)]
                s = pend.pop(i)
                pt = ptp.tile([P, 2, W], bf16, tag="pt", name=f"pt_{i}")
                nc.scalar.activation(pt[:], s[:, :, 0:W], Exp, scale=SCALE)
                if i + 1 < len(gsteps):
                    pend[i + 1] = emit_qk(*gsteps[i + 1], i + 1)
                for w in (0, 1):
                    nc.tensor.matmul(
                        o_ps[w][:],
                        lhsT=v[:, kc, :],
                        rhs=pt[:, w, :],
                        start=(kc == 0), stop=(kc == nkc - 1))
                if kc == 0:
                    nc.vector.tensor_copy(out=acc[:], in_=pt[:])
                else:
                    nc.vector.tensor_tensor(out=acc[:], in0=acc[:], in1=pt[:],
                                            op=Add)
                if kc != nkc - 1:
                    continue

                # ---- half tail: cross-partition row sums + stores ----
                for w in (0, 1):
                    nc.tensor.matmul(
                        sm_ps[0:1, w, 0:W], lhsT=ones_b[:], rhs=acc[:, w, :],
                        start=True, stop=True)
                for w in (0, 1):
                    o_sb = otp.tile([P, W], f32, tag="osb",
                                    name=f"osb_{p}_{h}_{w}")
                    nc.vector.tensor_copy(out=o_sb[:], in_=o_ps[w][:])
                    nc.sync.dma_start(
                        out_d[p][:, h * QH + w * W: h * QH + (w + 1) * W],
                        o_sb[:])
                sm_sb = smp.tile([1, 2, W], f32, tag="smsb",
                                 name=f"smsb_{p}_{h}")
                nc.vector.tensor_copy(out=sm_sb[:], in_=sm_ps[0:1, :, 0:W])
                nc.sync.dma_start(sums_d[p, h * QH:(h + 1) * QH], sm_sb[:])

    nc.compile()
    return nc


_PROG_CACHE: dict = {}


def _get_program(nkc: int = 13) -> bacc.Bacc:
    if nkc not in _PROG_CACHE:
        _PROG_CACHE[nkc] = build_program(HPC, nkc)
    return _PROG_CACHE[nkc]


def nkc_for_mask(attn_mask) -> int:
    mf = np.asarray(attn_mask) > 0
    valid = np.nonzero(mf.any(axis=0))[0]
    maxlen = int(valid[-1]) + 1 if valid.size else 1
    return min(16, max(8, -(-maxlen // 128)))


def make_in_maps(query, key, value, attn_mask):
    nkc = nkc_for_mask(attn_mask)
    LQ = LK = nkc * 128
    qT = np.asarray(query, np.float32).transpose(0, 2, 3, 1)[:, :, :, :LQ]
    kT = np.asarray(key, np.float32).transpose(0, 2, 3, 1)       # [B, H, D, S]
    v = np.asarray(value, np.float32).transpose(0, 2, 1, 3)      # [B, H, S, D]
    mf = (np.asarray(attn_mask) > 0).astype(np.float32)          # [B, S]
    kTz = (kT * mf[:, None, None, :])[:, :, :, :LK]
    vz = (v * mf[:, None, :, None])[:, :, :LK, :].astype(ml_dtypes.bfloat16)
    in_maps = []
    for c in range(N_CORES):
        b, h0 = c // CORES_PER_BATCH, (c % CORES_PER_BATCH) * HPC
        in_maps.append({
            "q": np.ascontiguousarray(qT[b, h0:h0 + HPC]),
            "k": np.ascontiguousarray(kTz[b, h0:h0 + HPC]),
            "v": np.ascontiguousarray(vz[b, h0:h0 + HPC]),
        })
    return in_maps, (mf, nkc)


def assemble_output(results, aux):
    mf, nkc = aux
    LQ = LK = nkc * 128
    # masked keys inside the computed window contribute exp(0)=1 to the sums
    mcount = (LK - mf[:, :LK].sum(axis=1)).astype(np.float32)    # [B]
    out = np.zeros((B, S, H * D), np.float32)
    for c in range(N_CORES):
        b, h0 = c // CORES_PER_BATCH, (c % CORES_PER_BATCH) * HPC
        oT = results[c]["out"]                                   # [HPC, D, LQ]
        sums = results[c]["sums_out"] - mcount[b]                # [HPC, LQ]
        with np.errstate(divide="ignore", invalid="ignore"):
            scale = np.where(mf[b][None, :LQ] > 0, 1.0 / sums, 0.0)
        o = oT * scale[:, None, :]                               # [HPC, D, LQ]
        for i in range(HPC):
            out[b, :LQ, (h0 + i) * D:(h0 + i + 1) * D] = o[i].T
    for b in range(B):
        if mf[b].sum() == 0.0:                                   # degenerate
            out[b] = 0.0
    return out


def kernel(query, key, value, attn_mask):
    in_maps, aux = make_in_maps(query, key, value, attn_mask)
    nc = _get_program(aux[1])
    res = run_bass_kernel_spmd(nc, in_maps, list(range(N_CORES)))
    return assemble_output(res.results, aux)
